# revision 3
# baseline (speedup 1.0000x reference)
"""XNOR-Net++ 3x3 conv (sign(x) (*) sign(w) * alpha*beta*gamma) on 8 TRN2 NeuronCores.

Strategy: data-parallel over batch (32 -> 4 per core), weights/scales replicated.
Per core: binarize x and w to bf16 (+-1 exact), build a zero-padded sign image in
SBUF, run the 3x3 conv as 18 accumulating 128x128x448 matmuls per output tile
(2 cin blocks x 9 taps), scale by alpha (per-channel, ACT) and beta*gamma
(per-pixel broadcast map, DVE), DMA out fp32.
"""

from contextlib import ExitStack

import numpy as np

import concourse.bacc as bacc
import concourse.bass as bass
import concourse.mybir as mybir
import concourse.tile as tile
from concourse import masks
from concourse.bass_utils import run_bass_kernel_spmd

N_CORES = 8
B, C, H, KS = 32, 256, 56, 3
P = 128
CB = C // P  # input-channel blocks (2)
OB = C // P  # output-channel blocks (2)
HP = H + 2   # padded image side (58)
R = 8        # output rows per matmul tile
T = H // R   # row tiles per image (7)
NT = R * H   # moving free dim per matmul (448)
HW = H * H   # pixels per image (3136)

F32 = mybir.dt.float32
BF16 = mybir.dt.bfloat16


def build_conv(tc, out_ap, x_ap, w_ap, a_ap, b_ap, g_ap, BL):
    nc = tc.nc
    with ExitStack() as ctx:
        const_pool = ctx.enter_context(tc.tile_pool(name="const", bufs=1))
        wpool = ctx.enter_context(tc.tile_pool(name="w", bufs=1))
        xpool = ctx.enter_context(tc.tile_pool(name="x", bufs=2))
        imgpool = ctx.enter_context(tc.tile_pool(name="img", bufs=2))
        psumpool = ctx.enter_context(tc.tile_pool(name="psum", bufs=4, space="PSUM"))
        tpool = ctx.enter_context(tc.tile_pool(name="tmp", bufs=4))
        opool = ctx.enter_context(tc.tile_pool(name="o", bufs=4))

        ident = const_pool.tile([P, P], BF16, name="ident")
        masks.make_identity(nc, ident)

        # ---- weights: load, binarize, transpose to [cin_low, cb, tap, ob, cout] ----
        w_f32 = wpool.tile([P, OB, C * KS * KS], F32, name="w_f32")
        nc.sync.dma_start(
            w_f32, w_ap.rearrange("(ob p) i ky kx -> p ob (i ky kx)", p=P)
        )
        w_sgn = wpool.tile([P, OB, C * KS * KS], BF16, name="w_sgn")
        nc.scalar.sign(w_sgn, w_f32)
        # view: [o_low, ob, tap, i]
        w_view = w_sgn.rearrange("p ob (i kk) -> p ob kk i", kk=KS * KS)

        wT = wpool.tile([P, CB, KS * KS, OB, P], BF16, name="wT")
        for ob in range(OB):
            for ib in range(CB):
                for kk in range(KS * KS):
                    pt = psumpool.tile([P, P], BF16, name="pt", tag="pt", bufs=2)
                    nc.tensor.transpose(
                        pt, w_view[:, ob, kk, ib * P : (ib + 1) * P], ident
                    )
                    nc.scalar.copy(wT[:, ib, kk, ob, :], pt)

        # ---- scales ----
        a_t = const_pool.tile([P, OB], F32, name="a_t")
        nc.sync.dma_start(a_t, a_ap.rearrange("(ob p) u v -> p (ob u v)", p=P))
        b_t = const_pool.tile([1, H], F32, name="b_t")
        nc.sync.dma_start(b_t, b_ap[0:1, :, 0])
        g_t = const_pool.tile([1, H], F32, name="g_t")
        nc.sync.dma_start(g_t, g_ap[0:1, 0, :])

        # bg_row[0, i*56+j] = beta[i] * gamma[j] — one DVE op with step-0 broadcast reads
        bg_row = const_pool.tile([1, HW], F32, name="bg_row")
        b_rep = b_t[0:1, :].unsqueeze(2).to_broadcast((1, H, H))
        g_rep = g_t[0:1, :].unsqueeze(1).to_broadcast((1, H, H))
        nc.vector.tensor_mul(
            bg_row.rearrange("a (i j) -> a i j", i=H), b_rep, g_rep
        )
        ones_t = const_pool.tile([1, P], F32, name="ones_t")
        nc.gpsimd.memset(ones_t, 1.0)
        # broadcast to all 128 partitions via K=1 matmul
        bg_bcast = const_pool.tile([P, HW], F32, name="bg_bcast")
        for t in range(T):
            sl = slice(t * NT, (t + 1) * NT)
            bgp = psumpool.tile([P, NT], F32, name="bgp", tag="bgp", bufs=2)
            nc.tensor.matmul(bgp, ones_t, bg_row[0:1, sl], start=True, stop=True)
            nc.scalar.copy(bg_bcast[:, sl], bgp)

        # ---- main loop over local batches ----
        x_v = x_ap.rearrange("b (cb p) h w -> b p cb (h w)", p=P)
        out_v = out_ap.rearrange("b (ob p) h w -> b ob p (h w)", p=P)
        n_acc = CB * KS * KS  # 18
        for b in range(BL):
            x_t = xpool.tile([P, CB, HW], F32, name="x_t")
            nc.sync.dma_start(x_t, x_v[b])
            img = imgpool.tile([P, CB, HP, HP], BF16, name="img")
            nc.gpsimd.memset(img, 0.0)
            nc.scalar.sign(
                img[:, :, 1 : H + 1, 1 : H + 1],
                x_t.rearrange("p cb (h w) -> p cb h w", h=H),
            )
            for ob in range(OB):
                for t in range(T):
                    ps = psumpool.tile([P, NT], F32, name="cps", tag="cps", bufs=4)
                    mi = 0
                    for cb in range(CB):
                        for ky in range(KS):
                            for kx in range(KS):
                                rhs = img[:, cb, t * R + ky : t * R + ky + R, kx : kx + H]
                                nc.tensor.matmul(
                                    ps,
                                    wT[:, cb, ky * KS + kx, ob, :],
                                    rhs,
                                    start=(mi == 0),
                                    stop=(mi == n_acc - 1),
                                )
                                mi += 1
                    sl = slice(t * NT, (t + 1) * NT)
                    tmp = tpool.tile([P, NT], F32, name="tmp")
                    nc.scalar.mul(tmp, ps, a_t[:, ob : ob + 1])
                    ot = opool.tile([P, NT], F32, name="ot")
                    nc.vector.tensor_mul(ot, tmp, bg_bcast[:, sl])
                    nc.sync.dma_start(out_v[b, ob][:, sl], ot)


def build_nc(BL):
    nc = bacc.Bacc("TRN2", target_bir_lowering=False, debug=False)
    x = nc.dram_tensor("x", [BL, C, H, H], F32, kind="ExternalInput")
    w = nc.dram_tensor("weight", [C, C, KS, KS], F32, kind="ExternalInput")
    a = nc.dram_tensor("alpha", [C, 1, 1], F32, kind="ExternalInput")
    be = nc.dram_tensor("beta", [1, H, 1], F32, kind="ExternalInput")
    g = nc.dram_tensor("gamma", [1, 1, H], F32, kind="ExternalInput")
    o = nc.dram_tensor("out", [BL, C, H, H], F32, kind="ExternalOutput")
    with tile.TileContext(nc) as tc:
        build_conv(tc, o.ap(), x.ap(), w.ap(), a.ap(), be.ap(), g.ap(), BL)
    nc.compile()
    return nc


_nc_cache = {}


def _get_nc(BL):
    if BL not in _nc_cache:
        _nc_cache[BL] = build_nc(BL)
    return _nc_cache[BL]


def kernel(x, weight, alpha, beta, gamma):
    x = np.ascontiguousarray(np.asarray(x, dtype=np.float32))
    weight = np.ascontiguousarray(np.asarray(weight, dtype=np.float32))
    alpha = np.ascontiguousarray(np.asarray(alpha, dtype=np.float32))
    beta = np.ascontiguousarray(np.asarray(beta, dtype=np.float32))
    gamma = np.ascontiguousarray(np.asarray(gamma, dtype=np.float32))

    BL = B // N_CORES
    nc = _get_nc(BL)
    xs = x.reshape(N_CORES, BL, C, H, H)
    in_maps = [
        {"x": xs[c], "weight": weight, "alpha": alpha, "beta": beta, "gamma": gamma}
        for c in range(N_CORES)
    ]
    res = run_bass_kernel_spmd(nc, in_maps, list(range(N_CORES)))
    return np.concatenate([r["out"] for r in res.results], axis=0)
